# revision 1
# baseline (speedup 1.0000x reference)
import sys

import numpy as np

sys.path.insert(0, "/opt/trn_rl_repo")

from concourse import bacc, bass, tile  # noqa: E402,F401
from concourse import mybir  # noqa: E402
from concourse.bass import broadcast_tensor_aps  # noqa: E402
from concourse.bass_utils import run_bass_kernel_spmd  # noqa: E402

N_CORES = 8
S = 8  # samples per core
C = 3
T = 9
H = W = 256
RC = 4  # rows per chunk (one SBUF partition holds one chunk)
NCH = H // RC  # 64 chunks per sample
RP = RC + 2  # row slots incl top/bottom halo
WP = W + 2  # col slots incl left/right reflect pad
F32 = mybir.dt.float32
F16 = mybir.dt.float16
NPROD = 4  # product ring depth
# center tap first (needs no halo rows / col pads), then row-halo-only
# taps, then col-pad taps, corners last: first mul waits only on the
# 6 channel DMAs + one sigma tap instead of all x DMAs + pads
TAP_ORDER = [4, 1, 7, 3, 5, 0, 2, 6, 8]


def build_nc():
    nc = bacc.Bacc()
    x_ext = nc.declare_dram_parameter("x", [S, C, H, W], F16, isOutput=False)
    sg_ext = nc.declare_dram_parameter("sigma", [S, T, H, W], F16, isOutput=False)
    out_ext = nc.declare_dram_parameter("out", [S, C, H, W], F32, isOutput=True)

    with tile.TileContext(nc) as tc:
        with tc.tile_pool(name="p", bufs=2) as pool:
            for stripe in range(S // 2):
                xt = pool.tile([128, C, RP, WP], F16)
                st = pool.tile([128, T, RC, W], F16)
                prods = [
                    pool.tile([128, C, RC, W], F16, name=f"prod{j}")
                    for j in range(NPROD)
                ]
                acc = pool.tile([128, C, RC, W], F16)
                den16 = pool.tile([128, RC, W], F16)
                den = pool.tile([128, 1, RC, W], F32)
                inv = pool.tile([128, 1, RC, W], F32)
                ot = pool.tile([128, C, RC, W], F32)

                for k in range(2):
                    s = 2 * stripe + k
                    pb = 64 * k
                    # disjoint partition halves -> run the two samples' DMAs
                    # on separate engine queues
                    eng = nc.sync if k == 0 else nc.scalar
                    xr = x_ext[s].rearrange("c (n r) w -> n c r w", r=RC)
                    sr = sg_ext[s].rearrange("t (n r) w -> n t r w", r=RC)
                    # main rows -> slots 1..RC, image cols -> slots 1..W
                    # (DMA APs are limited to 3 dims -> one DMA per channel)
                    for c in range(C):
                        eng.dma_start(
                            xt[pb : pb + 64, c, 1 : 1 + RC, 1 : 1 + W], xr[:, c]
                        )
                    # center tap's sigma right after the mains: its mul
                    # needs neither halos nor pads
                    eng.dma_start(
                        st[pb : pb + 64, TAP_ORDER[0]], sr[:, TAP_ORDER[0]]
                    )
                    # top halo row: chunks 1..63 read prev chunk row 3
                    eng.dma_start(
                        xt[pb + 1 : pb + 64, :, 0, 1 : 1 + W], xr[0:63, :, 3, :]
                    )
                    # chunk 0 top halo: reflect row 1
                    eng.dma_start(xt[pb : pb + 1, :, 0, 1 : 1 + W], xr[0:1, :, 1, :])
                    # bottom halo row: chunks 0..62 read next chunk row 0
                    eng.dma_start(xt[pb : pb + 63, :, 5, 1 : 1 + W], xr[1:64, :, 0, :])
                    # chunk 63 bottom halo: reflect row 254 (= chunk 63 row 2)
                    eng.dma_start(
                        xt[pb + 63 : pb + 64, :, 5, 1 : 1 + W], xr[63:64, :, 2, :]
                    )
                    # remaining sigma taps streamed in consumption order
                    for t in TAP_ORDER[1:]:
                        eng.dma_start(st[pb : pb + 64, t], sr[:, t])

                # column reflect pads: slot 0 <- image col 1 (slot 2),
                # slot WP-1 <- image col W-2 (slot WP-3)
                nc.scalar.copy(xt[:, :, :, 0:1], xt[:, :, :, 2:3])
                nc.scalar.copy(xt[:, :, :, WP - 1 : WP], xt[:, :, :, WP - 3 : WP - 2])

                # All compute on DVE: gpsimd touching recycled pool buffers
                # faults HW (NRT_EXEC_UNIT_UNRECOVERABLE 101). fp16 keeps
                # DVE in 2x_1p perf mode.
                with nc.allow_low_precision(reason="fp16 kernel"):
                    for j, t in enumerate(TAP_ORDER):
                        di, dj = t // 3, t % 3
                        xs = xt[:, :, di : di + RC, dj : dj + W]
                        sg = st[:, t : t + 1]
                        a, b = broadcast_tensor_aps(xs, sg)
                        nc.vector.tensor_mul(prods[j % NPROD][:], a, b)
                        if j == 1:
                            nc.vector.tensor_add(acc[:], prods[0][:], prods[1][:])
                        elif j > 1:
                            nc.vector.tensor_add(
                                acc[:], acc[:], prods[j % NPROD][:]
                            )

                    nc.vector.tensor_add(den16[:], st[:, 0], st[:, 1])
                    for t in range(2, T - 1):
                        nc.vector.tensor_add(den16[:], den16[:], st[:, t])
                nc.vector.tensor_add(den[:, 0], den16[:], st[:, T - 1])
                # ~5x faster than reciprocal(); ~18 correct bits >> fp16
                # noise floor, den in [0.8, 9] so no edge cases
                nc.vector.reciprocal_approx_fast(inv[:, 0], den[:, 0])

                # normalize + store per channel: out DMA of channel c starts
                # while channel c+1 is still normalizing (shrinks the tail)
                for c in range(C):
                    nc.vector.tensor_mul(ot[:, c], acc[:, c], inv[:, 0])
                    for k in range(2):
                        s = 2 * stripe + k
                        pb = 64 * k
                        eng = nc.sync if k == 0 else nc.scalar
                        orr = out_ext[s].rearrange("c (n r) w -> n c r w", r=RC)
                        eng.dma_start(orr[:, c], ot[pb : pb + 64, c])

    nc.finalize()
    return nc


_nc_cache = None


def _get_nc():
    global _nc_cache
    if _nc_cache is None:
        _nc_cache = build_nc()
    return _nc_cache


def _run(x, sigma, trace=False):
    x = np.ascontiguousarray(x).astype(np.float16)
    sigma = np.ascontiguousarray(sigma).astype(np.float16)
    nc = _get_nc()
    in_maps = [
        {"x": x[S * i : S * (i + 1)], "sigma": sigma[S * i : S * (i + 1)]}
        for i in range(N_CORES)
    ]
    res = run_bass_kernel_spmd(nc, in_maps, list(range(N_CORES)), trace=trace)
    out = np.concatenate([res.results[i]["out"] for i in range(N_CORES)], axis=0)
    return out.astype(np.float32, copy=False), res


def kernel(x, sigma):
    out, _ = _run(x, sigma)
    return out



# revision 2
# speedup vs baseline: 1.3329x; 1.3329x over previous
import sys

import numpy as np

sys.path.insert(0, "/opt/trn_rl_repo")

from concourse import bacc, bass, tile  # noqa: E402,F401
from concourse import mybir  # noqa: E402
from concourse.bass import broadcast_tensor_aps  # noqa: E402
from concourse.bass_utils import run_bass_kernel_spmd  # noqa: E402

N_CORES = 8
S = 8  # samples per core
C = 3
T = 9
H = W = 256
RC = 4  # rows per chunk (one SBUF partition holds one chunk)
NCH = H // RC  # 64 chunks per sample
RP = RC + 2  # row slots incl top/bottom halo
WP = W + 2  # col slots incl left/right reflect pad
F32 = mybir.dt.float32
F16 = mybir.dt.float16
NPROD = 4  # product ring depth
# dj=0/2 taps first (need only col pads), dj=1 taps (1,4,7) last so the
# ScalarE-made shifted copy xt1 is ready; the pipelined norm of the
# previous stripe is inserted after 4 muls, the reciprocal after 6.
TAP_ORDER = [0, 2, 6, 8, 3, 1, 5, 4, 7]
NORM_AT = 4  # insert prev-stripe normalize before this mul index
RECIP_AT = 6  # insert this stripe's reciprocal before this mul index


def build_nc():
    nc = bacc.Bacc()
    x_ext = nc.declare_dram_parameter("x", [S, C, H, W], F16, isOutput=False)
    sg_ext = nc.declare_dram_parameter("sigma", [S, T, H, W], F16, isOutput=False)
    id_ext = nc.declare_dram_parameter("ident", [128, 128], F16, isOutput=False)
    out_ext = nc.declare_dram_parameter("out", [S, C, H, W], F16, isOutput=True)

    with tile.TileContext(nc) as tc:
        with (
            tc.tile_pool(name="const", bufs=1) as cpool,
            tc.tile_pool(name="p", bufs=2) as pool,
            tc.tile_pool(name="prods", bufs=NPROD) as ppool,
            tc.tile_pool(name="ps", bufs=1, space="PSUM") as psp,
        ):
            ident = cpool.tile([128, 128], F16)
            nc.sync.dma_start(ident[:], id_ext[:])

            prev = None  # (acc16, inv16, ot, stripe) pending normalize+store

            def norm_store(prev):
                acc16, inv16, ot, stripe = prev
                # normalize: out_c = acc_c * inv (all fp16 SBUF -> 2x mode)
                with nc.allow_low_precision(reason="fp16 kernel"):
                    for c in range(C):
                        nc.vector.tensor_mul(ot[:, c], acc16[:, c], inv16[:])
                # store on the ACT hwdge ring (sync ring carries the inputs)
                for k in range(2):
                    s = 2 * stripe + k
                    pb = 64 * k
                    orr = out_ext[s].rearrange("c (n r) w -> n c r w", r=RC)
                    for c in range(C):
                        nc.scalar.dma_start(orr[:, c], ot[pb : pb + 64, c])

            for stripe in range(S // 2):
                xt = pool.tile([128, C, RP, WP], F16)
                xt1 = pool.tile([128, C, RP, W], F16)  # xt shifted left 1 col
                st = pool.tile([128, T, RC, W], F16)
                acc16 = pool.tile([128, C, RC, W], F16)
                inv32 = pool.tile([128, RC, W], F32)
                inv16 = pool.tile([128, RC, W], F16)
                ot = pool.tile([128, C, RC, W], F16)
                psum_acc = psp.tile([128, C, RC * W], F32)  # 6 banks
                psum_den = psp.tile([128, RC, W], F32)  # 2 banks

                for k in range(2):
                    s = 2 * stripe + k
                    pb = 64 * k
                    xr = x_ext[s].rearrange("c (n r) w -> n c r w", r=RC)
                    sr = sg_ext[s].rearrange("t (n r) w -> n t r w", r=RC)
                    # main rows -> slots 1..RC, image cols -> slots 1..W
                    # (DMA APs are limited to 3 dims -> one DMA per channel)
                    for c in range(C):
                        nc.sync.dma_start(
                            xt[pb : pb + 64, c, 1 : 1 + RC, 1 : 1 + W], xr[:, c]
                        )
                    # top halo row: chunks 1..63 read prev chunk row 3
                    nc.sync.dma_start(
                        xt[pb + 1 : pb + 64, :, 0, 1 : 1 + W], xr[0:63, :, 3, :]
                    )
                    # chunk 0 top halo: reflect row 1
                    nc.sync.dma_start(
                        xt[pb : pb + 1, :, 0, 1 : 1 + W], xr[0:1, :, 1, :]
                    )
                    # bottom halo row: chunks 0..62 read next chunk row 0
                    nc.sync.dma_start(
                        xt[pb : pb + 63, :, 5, 1 : 1 + W], xr[1:64, :, 0, :]
                    )
                    # chunk 63 bottom halo: reflect row 254 (= chunk 63 row 2)
                    nc.sync.dma_start(
                        xt[pb + 63 : pb + 64, :, 5, 1 : 1 + W], xr[63:64, :, 2, :]
                    )
                    for t in range(T):
                        nc.sync.dma_start(st[pb : pb + 64, t], sr[:, t])

                # column reflect pads: slot 0 <- image col 1 (slot 2),
                # slot WP-1 <- image col W-2 (slot WP-3). Tiny -> DVE.
                nc.vector.tensor_copy(xt[:, :, :, 0:1], xt[:, :, :, 2:3])
                nc.vector.tensor_copy(
                    xt[:, :, :, WP - 1 : WP], xt[:, :, :, WP - 3 : WP - 2]
                )
                # dj=1 taps start at a 2-byte offset which drops DVE
                # tensor_tensor to 1x mode; give them a 4B-aligned copy.
                nc.scalar.copy(xt1[:], xt[:, :, :, 1 : 1 + W])

                # denominator on PE: psum_den[:, rows] += I.T @ sigma_t
                # (one matmul output must stay within one 2KB PSUM bank ->
                # split the 4 rows into two 2-row groups)
                for h in range(2):
                    for t in range(T):
                        nc.tensor.matmul(
                            psum_den[:, 2 * h : 2 * h + 2, :],
                            ident[:],
                            st[:, t, 2 * h : 2 * h + 2, :],
                            start=(t == 0),
                            stop=(t == T - 1),
                        )

                with nc.allow_low_precision(reason="fp16 kernel"):
                    for j, t in enumerate(TAP_ORDER):
                        if j == NORM_AT and prev is not None:
                            norm_store(prev)
                        if j == RECIP_AT:
                            # ~18 correct bits >> fp16 noise floor; den in
                            # [0.8, 9] so no edge cases
                            nc.vector.reciprocal_approx_fast(
                                inv32[:], psum_den[:]
                            )
                            nc.scalar.copy(inv16[:], inv32[:])
                        di, dj = t // 3, t % 3
                        if dj == 1:
                            xs = xt1[:, :, di : di + RC, 0:W]
                        else:
                            xs = xt[:, :, di : di + RC, dj : dj + W]
                        sg = st[:, t : t + 1]
                        a, b = broadcast_tensor_aps(xs, sg)
                        prod = ppool.tile([128, C, RC, W], F16)
                        nc.vector.tensor_mul(prod[:], a, b)
                        # accumulate this tap into PSUM on the PE
                        for c in range(C):
                            for h in range(2):
                                nc.tensor.matmul(
                                    psum_acc[
                                        :, c, 512 * h : 512 * (h + 1)
                                    ],
                                    ident[:],
                                    prod[:, c, 2 * h : 2 * h + 2, :],
                                    start=(j == 0),
                                    stop=(j == T - 1),
                                )

                    # drain PSUM acc -> SBUF fp16 on ScalarE (frees DVE)
                    nc.scalar.copy(acc16[:], psum_acc[:])

                prev = (acc16, inv16, ot, stripe)

            norm_store(prev)

    nc.finalize()
    return nc


_nc_cache = None


def _get_nc():
    global _nc_cache
    if _nc_cache is None:
        _nc_cache = build_nc()
    return _nc_cache


def _run(x, sigma, trace=False):
    x = np.ascontiguousarray(x).astype(np.float16)
    sigma = np.ascontiguousarray(sigma).astype(np.float16)
    ident = np.eye(128, dtype=np.float16)
    nc = _get_nc()
    in_maps = [
        {
            "x": x[S * i : S * (i + 1)],
            "sigma": sigma[S * i : S * (i + 1)],
            "ident": ident,
        }
        for i in range(N_CORES)
    ]
    res = run_bass_kernel_spmd(nc, in_maps, list(range(N_CORES)), trace=trace)
    out = np.concatenate([res.results[i]["out"] for i in range(N_CORES)], axis=0)
    return out.astype(np.float32, copy=False), res


def kernel(x, sigma):
    out, _ = _run(x, sigma)
    return out


# revision 7
# speedup vs baseline: 1.3497x; 1.0126x over previous
import sys

import numpy as np

sys.path.insert(0, "/opt/trn_rl_repo")

from concourse import bacc, bass, tile  # noqa: E402,F401
from concourse import mybir  # noqa: E402
from concourse.bass import AP, broadcast_tensor_aps  # noqa: E402
from concourse.bass_utils import run_bass_kernel_spmd  # noqa: E402

N_CORES = 8
S = 8  # samples per core
C = 3
T = 9
H = W = 256
RC = 4  # rows per chunk (one SBUF partition holds one chunk)
NCH = H // RC  # 64 chunks per sample
RP = RC + 2  # row slots incl top/bottom halo
WP = W + 2  # col slots incl left/right reflect pad
F32 = mybir.dt.float32
F16 = mybir.dt.float16
NPROD = 6  # product ring depth (PE lags DVE by several taps)
XFLAT = RP * 0 + (H + 2) * C * WP  # 258*3*258 elems per padded sample
XROW = C * WP  # 774: one padded row (all channels)


def build_nc():
    nc = bacc.Bacc()
    # x arrives host-side transposed to [H+2, C, W+2] (reflect-padded) and
    # flattened, so one overlapping-window DMA per sample covers mains,
    # halo rows and col pads; a second DMA at +1 elem gives the 4B-aligned
    # odd-column copy (dj=1 taps would otherwise drop DVE to 1x mode).
    x_ext = nc.declare_dram_parameter("x", [S, XFLAT], F16, isOutput=False)
    sg_ext = nc.declare_dram_parameter("sigma", [S, T, H, W], F16, isOutput=False)
    id_ext = nc.declare_dram_parameter("ident", [128, 128], F16, isOutput=False)
    # output in [H, C, W] layout -> contiguous per-chunk rows, one DMA/sample
    out_ext = nc.declare_dram_parameter("out", [S, H, C, W], F16, isOutput=True)

    with tile.TileContext(nc) as tc:
        with (
            tc.tile_pool(name="const", bufs=1) as cpool,
            tc.tile_pool(name="p", bufs=2) as pool,
            tc.tile_pool(name="prods", bufs=NPROD) as ppool,
            tc.tile_pool(name="ps", bufs=1, space="PSUM") as psp,
        ):
            ident = cpool.tile([128, 128], F16)
            nc.sync.dma_start(ident[:], id_ext[:])

            prev = None  # pending (acc16, inv32, inv16, ot, stripe)

            def finish_prev(prev):
                acc16, inv32, inv16, ot, stripe = prev
                # fp32 -> fp16 so the normalize runs in DVE 2x mode
                nc.scalar.copy(
                    inv16[:, :, 0, :], inv32[:].rearrange("p (r w) -> p r w", r=RC)
                )
                with nc.allow_low_precision(reason="fp16 kernel"):
                    a, b = broadcast_tensor_aps(acc16[:], inv16[:])
                    nc.vector.tensor_mul(ot[:], a, b)
                for k in range(2):
                    s = 2 * stripe + k
                    pb = 64 * k
                    nc.scalar.dma_start(
                        out_ext[s].rearrange("(n r) c w -> n (r c w)", r=RC),
                        ot[pb : pb + 64].rearrange("n r c w -> n (r c w)"),
                    )

            for stripe in range(S // 2):
                xt = pool.tile([128, RP, C, WP], F16)
                xt1 = pool.tile([128, RP, C, WP], F16)  # shifted 1 col left
                st = pool.tile([128, T, RC, W], F16)
                acc16 = pool.tile([128, RC, C, W], F16)
                den32 = pool.tile([128, RC * W], F32)
                inv32 = pool.tile([128, RC * W], F32)
                inv16 = pool.tile([128, RC, 1, W], F16)
                ot = pool.tile([128, RC, C, W], F16)
                psum_acc = psp.tile([128, RC * C * W], F32)  # 6 banks
                psum_den = psp.tile([128, RC * W], F32)  # 2 banks

                for k in range(2):
                    s = 2 * stripe + k
                    pb = 64 * k
                    xs_ap = x_ext[s]
                    # chunk n reads padded rows 4n..4n+5 (6-row window,
                    # stride 4 rows): mains + halos + col pads in one DMA
                    nc.sync.dma_start(
                        xt[pb : pb + 64].rearrange("n r c w -> n (r c w)"),
                        AP(xs_ap.tensor, xs_ap.offset, [[4 * XROW, 64], [1, RP * XROW]]),
                    )
                    # same window shifted +1 element (drop last 2 so the
                    # final chunk stays in bounds; dj=1 taps never read them)
                    nc.sync.dma_start(
                        xt1[pb : pb + 64].rearrange("n r c w -> n (r c w)")[
                            :, : RP * XROW - 2
                        ],
                        AP(
                            xs_ap.tensor,
                            xs_ap.offset + 1,
                            [[4 * XROW, 64], [1, RP * XROW - 2]],
                        ),
                    )
                    sg_ap = sg_ext[s]
                    nc.sync.dma_start(
                        st[pb : pb + 64].rearrange("n t r w -> n t (r w)"),
                        AP(
                            sg_ap.tensor,
                            sg_ap.offset,
                            [[RC * W, 64], [H * W, T], [1, RC * W]],
                        ),
                    )

                # ---- PE: one continuous burst (den, then acc per tap) so
                # the tensor engine p-state ramps to full clock ----
                stf = st[:].rearrange("p t r w -> p t (r w)")
                for t in range(T):
                    for h in range(2):
                        nc.tensor.matmul(
                            psum_den[:, 512 * h : 512 * (h + 1)],
                            ident[:],
                            stf[:, t, 512 * h : 512 * (h + 1)],
                            start=(t == 0),
                            stop=(t == T - 1),
                        )

                # denominator to SBUF early so psum_den frees fast; the
                # reciprocal then reads SBUF and never stalls the PE
                nc.scalar.copy(den32[:], psum_den[:])

                with nc.allow_low_precision(reason="fp16 kernel"):
                    for j in range(T):
                        di, dj = j // 3, j % 3
                        src = xt1 if dj == 1 else xt
                        off = dj - 1 if dj == 1 else dj
                        xs = src[:, di : di + RC, :, off : off + W]
                        sg = st[:, j].unsqueeze(2)
                        a, b = broadcast_tensor_aps(xs, sg)
                        prod = ppool.tile([128, RC, C, W], F16)
                        nc.vector.tensor_mul(prod[:], a, b)
                        mv = prod[:].rearrange("p r c w -> p (r c w)")
                        for k in range(6):
                            nc.tensor.matmul(
                                psum_acc[:, 512 * k : 512 * (k + 1)],
                                ident[:],
                                mv[:, 512 * k : 512 * (k + 1)],
                                start=(j == 0),
                                stop=(j == T - 1),
                            )

                    # drain PSUM acc -> SBUF fp16 on ScalarE
                    nc.scalar.copy(
                        acc16[:].rearrange("p r c w -> p (r c w)"), psum_acc[:]
                    )

                if prev is not None:
                    finish_prev(prev)
                nc.vector.reciprocal_approx_fast(inv32[:], den32[:])
                prev = (acc16, inv32, inv16, ot, stripe)

            finish_prev(prev)

    nc.finalize()
    return nc


_nc_cache = None


def _get_nc():
    global _nc_cache
    if _nc_cache is None:
        _nc_cache = build_nc()
    return _nc_cache


def _run(x, sigma, trace=False):
    x = np.ascontiguousarray(x).astype(np.float16)
    sigma = np.ascontiguousarray(sigma).astype(np.float16)
    # [N, C, H, W] -> [N, H, C, W], reflect-pad H and W by 1, flatten
    xp = np.pad(
        x.transpose(0, 2, 1, 3), ((0, 0), (1, 1), (0, 0), (1, 1)), mode="reflect"
    )
    xp = np.ascontiguousarray(xp).reshape(x.shape[0], -1)
    ident = np.eye(128, dtype=np.float16)
    nc = _get_nc()
    in_maps = [
        {
            "x": xp[S * i : S * (i + 1)],
            "sigma": sigma[S * i : S * (i + 1)],
            "ident": ident,
        }
        for i in range(N_CORES)
    ]
    res = run_bass_kernel_spmd(nc, in_maps, list(range(N_CORES)), trace=trace)
    out = np.concatenate([res.results[i]["out"] for i in range(N_CORES)], axis=0)
    # device wrote [S, H, C, W]; back to [N, C, H, W]
    out = out.transpose(0, 2, 1, 3)
    return np.ascontiguousarray(out, dtype=np.float32), res


def kernel(x, sigma):
    out, _ = _run(x, sigma)
    return out


# revision 9
# speedup vs baseline: 1.5559x; 1.1528x over previous
import sys

import numpy as np

sys.path.insert(0, "/opt/trn_rl_repo")

from concourse import bacc, bass, tile  # noqa: E402,F401
from concourse import mybir  # noqa: E402
from concourse.bass import AP, broadcast_tensor_aps  # noqa: E402
from concourse.bass_utils import run_bass_kernel_spmd  # noqa: E402

N_CORES = 8
S = 8  # samples per core
C = 3
T = 9
H = W = 256
RC = 4  # rows per chunk (one SBUF partition holds one chunk)
NCH = H // RC  # 64 chunks per sample
RP = RC + 2  # row slots incl top/bottom halo
WP = W + 2  # col slots incl left/right reflect pad
F32 = mybir.dt.float32
F16 = mybir.dt.float16
NPROD = 6  # product ring depth (PE lags DVE by several taps)
XROW = C * WP  # 774: one padded row (all channels)
XFLAT = (H + 2) * XROW  # elems per padded sample
# dj=1 taps (1,4,7) last: they read the ScalarE-made shifted copy xt1.
# norm of the previous stripe is woven in after 3 muls, the reciprocal
# right after (hides the PE->ScalarE->DVE tail of the previous stripe).
TAP_ORDER = [0, 2, 3, 5, 6, 8, 1, 4, 7]
NORM_AT = 3
RECIP_AT = 4


def build_nc():
    nc = bacc.Bacc()
    # x arrives host-side transposed to [H+2, C, W+2] (reflect-padded) and
    # flattened, so one overlapping-window DMA per sample covers mains,
    # halo rows and col pads.
    x_ext = nc.declare_dram_parameter("x", [S, XFLAT], F16, isOutput=False)
    sg_ext = nc.declare_dram_parameter("sigma", [S, T, H, W], F16, isOutput=False)
    id_ext = nc.declare_dram_parameter("ident", [128, 128], F16, isOutput=False)
    # output in [H, C, W] layout -> contiguous per-chunk rows, one DMA/sample
    out_ext = nc.declare_dram_parameter("out", [S, H, C, W], F16, isOutput=True)

    def dma_x(stripe, xt):
        # x on the SP hwdge ring
        for k in range(2):
            s = 2 * stripe + k
            pb = 64 * k
            a = x_ext[s]
            # chunk n reads padded rows 4n..4n+5 (6-row overlapping window)
            nc.sync.dma_start(
                xt[pb : pb + 64].rearrange("n r c w -> n (r c w)"),
                AP(a.tensor, a.offset, [[4 * XROW, 64], [1, RP * XROW]]),
            )

    def dma_sigma(stripe, st):
        # sigma on the ACT hwdge ring
        for k in range(2):
            s = 2 * stripe + k
            pb = 64 * k
            a = sg_ext[s]
            nc.scalar.dma_start(
                st[pb : pb + 64].rearrange("n t r w -> n t (r w)"),
                AP(a.tensor, a.offset, [[RC * W, 64], [H * W, T], [1, RC * W]]),
            )

    with tile.TileContext(nc) as tc:
        with (
            tc.tile_pool(name="const", bufs=1) as cpool,
            tc.tile_pool(name="p", bufs=2) as pool,
            tc.tile_pool(name="prods", bufs=NPROD) as ppool,
            tc.tile_pool(name="ps", bufs=1, space="PSUM") as psp,
        ):
            ident = cpool.tile([128, 128], F16)
            nc.sync.dma_start(ident[:], id_ext[:])

            NS = S // 2
            xts = [
                pool.tile([128, RP, C, WP], F16, name=f"xt_{i}", bufs=1)
                for i in range(2)
            ]
            sts = [
                pool.tile([128, T, RC, W], F16, name=f"st_{i}", bufs=1)
                for i in range(2)
            ]
            # prefetch stripe 0 before entering the loop
            dma_x(0, xts[0])
            dma_sigma(0, sts[0])

            prev = None  # pending (acc16, inv16, ot, stripe)

            def norm_of(prev):
                acc16, inv16, ot, stripe = prev
                with nc.allow_low_precision(reason="fp16 kernel"):
                    a, b = broadcast_tensor_aps(acc16[:], inv16[:])
                    nc.vector.tensor_mul(ot[:], a, b)

            def store_of(prev):
                acc16, inv16, ot, stripe = prev
                for k in range(2):
                    s = 2 * stripe + k
                    pb = 64 * k
                    nc.scalar.dma_start(
                        out_ext[s].rearrange("(n r) c w -> n (r c w)", r=RC),
                        ot[pb : pb + 64].rearrange("n r c w -> n (r c w)"),
                    )

            for stripe in range(NS):
                xt = xts[stripe % 2]
                st = sts[stripe % 2]
                xt1 = pool.tile([128, RP, C, W], F16)  # xt shifted 1 col left
                acc16 = pool.tile([128, RC, C, W], F16)
                den32 = pool.tile([128, RC * W], F32)
                inv32 = pool.tile([128, RC * W], F32)
                inv16 = pool.tile([128, RC, 1, W], F16)
                ot = pool.tile([128, RC, C, W], F16)
                psum_acc = psp.tile([128, RC * C * W], F32)  # 6 banks
                psum_den = psp.tile([128, RC * W], F32)  # 2 banks

                # prefetch next stripe's inputs (queue-ordered ahead of the
                # ScalarE compute so the transfers overlap this stripe)
                if stripe + 1 < NS:
                    dma_x(stripe + 1, xts[(stripe + 1) % 2])
                    dma_sigma(stripe + 1, sts[(stripe + 1) % 2])

                # dj=1 taps start at a 2-byte offset which drops DVE
                # tensor_tensor to 1x mode; give them a 4B-aligned copy
                nc.scalar.copy(xt1[:], xt[:, :, :, 1 : 1 + W])

                # ---- PE: one continuous burst (den, then acc per tap) ----
                stf = st[:].rearrange("p t r w -> p t (r w)")
                for t in range(T):
                    for h in range(2):
                        nc.tensor.matmul(
                            psum_den[:, 512 * h : 512 * (h + 1)],
                            ident[:],
                            stf[:, t, 512 * h : 512 * (h + 1)],
                            start=(t == 0),
                            stop=(t == T - 1),
                        )

                # denominator to SBUF early; the reciprocal reads SBUF and
                # psum_den frees long before the next stripe needs it
                nc.scalar.copy(den32[:], psum_den[:])

                with nc.allow_low_precision(reason="fp16 kernel"):
                    for j, t in enumerate(TAP_ORDER):
                        if j == NORM_AT and prev is not None:
                            norm_of(prev)
                        if j == RECIP_AT:
                            # ~18 correct bits; den in [0.8, 9]: no edge cases
                            nc.vector.reciprocal_approx_fast(inv32[:], den32[:])
                        di, dj = t // 3, t % 3
                        if dj == 1:
                            xs = xt1[:, di : di + RC, :, 0:W]
                        else:
                            xs = xt[:, di : di + RC, :, dj : dj + W]
                        sg = st[:, t].unsqueeze(2)
                        a, b = broadcast_tensor_aps(xs, sg)
                        prod = ppool.tile([128, RC, C, W], F16)
                        nc.vector.tensor_mul(prod[:], a, b)
                        mv = prod[:].rearrange("p r c w -> p (r c w)")
                        for kk in range(6):
                            nc.tensor.matmul(
                                psum_acc[:, 512 * kk : 512 * (kk + 1)],
                                ident[:],
                                mv[:, 512 * kk : 512 * (kk + 1)],
                                start=(j == 0),
                                stop=(j == T - 1),
                            )

                    # fp32 -> fp16 so the normalize runs in DVE 2x mode
                    nc.scalar.copy(
                        inv16[:, :, 0, :],
                        inv32[:].rearrange("p (r w) -> p r w", r=RC),
                    )
                    # drain PSUM acc -> SBUF fp16 on ScalarE
                    nc.scalar.copy(
                        acc16[:].rearrange("p r c w -> p (r c w)"), psum_acc[:]
                    )

                if prev is not None:
                    store_of(prev)
                prev = (acc16, inv16, ot, stripe)

            norm_of(prev)
            store_of(prev)

    nc.finalize()
    return nc


_nc_cache = None


def _get_nc():
    global _nc_cache
    if _nc_cache is None:
        _nc_cache = build_nc()
    return _nc_cache


def _run(x, sigma, trace=False):
    x = np.ascontiguousarray(x).astype(np.float16)
    sigma = np.ascontiguousarray(sigma).astype(np.float16)
    # [N, C, H, W] -> [N, H, C, W], reflect-pad H and W by 1, flatten
    xp = np.pad(
        x.transpose(0, 2, 1, 3), ((0, 0), (1, 1), (0, 0), (1, 1)), mode="reflect"
    )
    xp = np.ascontiguousarray(xp).reshape(x.shape[0], -1)
    ident = np.eye(128, dtype=np.float16)
    nc = _get_nc()
    in_maps = [
        {
            "x": xp[S * i : S * (i + 1)],
            "sigma": sigma[S * i : S * (i + 1)],
            "ident": ident,
        }
        for i in range(N_CORES)
    ]
    res = run_bass_kernel_spmd(nc, in_maps, list(range(N_CORES)), trace=trace)
    out = np.concatenate([res.results[i]["out"] for i in range(N_CORES)], axis=0)
    # device wrote [S, H, C, W]; back to [N, C, H, W]
    out = out.transpose(0, 2, 1, 3)
    return np.ascontiguousarray(out, dtype=np.float32), res


def kernel(x, sigma):
    out, _ = _run(x, sigma)
    return out
